# revision 7
# baseline (speedup 1.0000x reference)
"""Trainium2 Bass kernel for the patch-GP conditional (conv GP layer).

Contract: kernel(**inputs) takes the FULL inputs (as produced by
setup_inputs()) and returns the FULL output (mean, var), each [N, P*G].

Math (equivalent to the reference conditional):
    Kuf[g,m,x] = var * exp(scale*(||z_m||^2 - 2 z_m.x_x + ||x_x||^2)),
                 scale = -0.5/ls^2
    fmean[g,x] = d_g^T Kuf[g,:,x],          d_g  = Kuu_g^{-1} q_mu[:,g]
    fvar[g,x]  = var - Kuf^T Q_g Kuf,       Q_g  = Kuu_g^{-1} - B B^T,
                                            B    = Kuu_g^{-1} tril(q_sqrt_g)
Since Q is symmetric the quadratic form uses the halved-triangle trick:
    kt^T Q kt = kt^T Lh kt,   Lh = 2*tril(Q,-1) + diag(Q)
so only the 6 lower-triangular 128x128 blocks of Lh are multiplied
(6 PE streams instead of 9 per column chunk).

The per-inducing-point exp bias (scale*||z||^2 + ln var) and the per-column
||x||^2 are folded into two extra contraction rows of the sq matmul
(K = 75+2 = 77), so the ACT uses an immediate scale/zero bias and no
host-side rescale of the outputs is needed.

Per core (X = 98*32 = 3136 columns, chunks of 512 + a 64 tail), per g:
    psq_mt  = zaug_mt^T @ xaug     (3 matmuls, K=77, one PSUM bank each)
    kt_mt   = exp(scale*psq_mt)    (3 ACTs, fp16)
    R       = Lh @ kt              (6 matmuls - lower-tri blocks)
    pm      = dblk^T kt            (3 matmuls, M=33: d in col 0, zero pad
                                    so PSUM partitions 0..32 are defined)
    pk_mt   = kt_mt .* R_mt        (3 DVE muls)
    pa      = pk0 + pk1            (GpSimd add)   pacc = pa + pk2 (DVE)
    pv      = ones^T pacc          (1 matmul -> PSUM partition 32)
    copy    = PSUM[0:33] -> SBUF   (1 Scalar copy grabs pm and pv rows)
The pv matmul and the output copy of chunk i are emitted during chunk
i+1 (software pipelining) so the strict-FIFO PE/Scalar queues never
stall on the DVE/GpSimd reduction chain. Inputs arrive as packed blob
DMAs; outputs DMA from the staging rows (prefix early, 64-wide tail
last to keep the drain short).
"""

import numpy as np

# Problem constants (hardcoded per the task contract).
H = 32
W = 32
C = 3
PH = 5
PW = 5
JITTER = 1e-6
N = 32
G = 2
M = 384
L = PH * PW * C  # 75
LA = L + 2  # augmented contraction rows (ones, xsq)
P = (H - PH + 1) * (W - PW + 1)  # 784
NCORES = 8
PLOC = P // NCORES  # 98
XL = PLOC * N  # 3136
CHW = 512
_FULL = [(i * CHW, CHW) for i in range(XL // CHW)]
_TINY = [(XL - XL % CHW, XL % CHW)]
CHUNKS_G = [_TINY + _FULL, _FULL + _TINY]
PRE = XL - XL % CHW  # 3072: prefix DMA boundary
MT = M // 128  # 3 partition tiles of the inducing dim
TRI = [(0, 0), (1, 0), (1, 1), (2, 0), (2, 1), (2, 2)]  # (mt, kt) lower blocks
DW = 33  # pm lhsT width: d + zero pad covering PSUM partitions 0..32
WARM_MM = 14

# blob77 column layout (fp16): [zaug G*M | xt XL]
ZCOLS = G * M  # 768
B77 = ZCOLS + XL  # 3904
# qblob column layout (fp16), g0 essentials first:
#   [q-g0 6*128 | dblk-g0 3*33 | ones | q-g1 6*128 | dblk-g1 3*33]
QG = len(TRI) * 128  # 768
Q_OFF = [0, QG + MT * DW + 1]
DV_OFF = [QG, QG + MT * DW + 1 + QG]
ONES_OFF = QG + MT * DW
QB = 2 * QG + 2 * MT * DW + 1

_CACHE = {}


def _ensure_concourse():
    try:
        import concourse  # noqa: F401
    except ImportError:
        import sys

        for p in ("/opt/trn_rl_repo", "/root/.axon_site/_ro/trn_rl_repo"):
            if p not in sys.path:
                sys.path.insert(0, p)


def _build(scale_imm: float):
    """Build + compile the single-core SPMD program (same NEFF on all cores)."""
    _ensure_concourse()
    from concourse import bacc, mybir, tile

    f32 = mybir.dt.float32
    f16 = mybir.dt.float16
    EXP = mybir.ActivationFunctionType.Exp

    nc = bacc.Bacc("TRN2", target_bir_lowering=False, debug=False)

    blob77 = nc.dram_tensor("blob77", [LA, B77], f16, kind="ExternalInput").ap()
    qblob = nc.dram_tensor("qblob", [128, QB], f16, kind="ExternalInput").ap()
    out = nc.dram_tensor("out", [2 * G, XL], f32, kind="ExternalOutput").ap()

    with tile.TileContext(nc) as tc:
        with (
            tc.tile_pool(name="const", bufs=1) as const,
            tc.tile_pool(name="work", bufs=2) as work,
            tc.tile_pool(name="ps", bufs=1, space="PSUM") as ps,
        ):
            # PE warmup: dependency-free matmuls issued while input DMAs are
            # in flight so the HAM clock gate reaches 8/8 before real work.
            wsrc = const.tile([128, 256], f16)
            nc.vector.memset(wsrc, 0.0)
            for _ in range(WARM_MM):
                wps = ps.tile([128, CHW], f32, tag="pr", name="wps", bufs=3)
                nc.tensor.matmul(wps[:, 0:256], wsrc[:, 0:128], wsrc)

            c77 = const.tile([LA, B77], f16)
            nc.sync.dma_start(out=c77[:, 0:ZCOLS], in_=blob77[:, 0:ZCOLS])
            nc.sync.dma_start(
                out=c77[:, ZCOLS + PRE : B77], in_=blob77[:, ZCOLS + PRE : B77]
            )
            nc.sync.dma_start(
                out=c77[:, ZCOLS : ZCOLS + CHW], in_=blob77[:, ZCOLS : ZCOLS + CHW]
            )
            qsb = const.tile([128, QB], f16)
            nc.sync.dma_start(out=qsb[:, 0 : Q_OFF[1]], in_=qblob[:, 0 : Q_OFF[1]])
            mid = ZCOLS + 3 * CHW
            nc.sync.dma_start(
                out=c77[:, ZCOLS + CHW : mid], in_=blob77[:, ZCOLS + CHW : mid]
            )
            nc.sync.dma_start(out=c77[:, mid:B77], in_=blob77[:, mid:B77])
            nc.sync.dma_start(out=qsb[:, Q_OFF[1] : QB], in_=qblob[:, Q_OFF[1] : QB])

            xaug = c77[:, ZCOLS:B77]
            ones = qsb[:, ONES_OFF : ONES_OFF + 1]

            stacc = [const.tile([DW, XL], f32, name=f"stacc{g}") for g in range(G)]

            def emit_tail(pend):
                """pv matmul + output copy for the previous chunk."""
                pg, poff, pcw, ppt, ppacc = pend
                psl = slice(poff, poff + pcw)
                nc.tensor.matmul(
                    ppt[32:33, :pcw], ones, ppacc[:, :pcw], skip_group_check=True
                )
                nc.scalar.copy(stacc[pg][:, psl], ppt[0:DW, :pcw])
                if poff + pcw == PRE:
                    lo, hi = (0, XL) if pg == 0 else (0, PRE)
                    nc.sync.dma_start(
                        out=out[pg : 2 * G : G, lo:hi],
                        in_=stacc[pg][0:DW:32, lo:hi],
                    )
                elif poff + pcw == XL and pg == G - 1:
                    nc.sync.dma_start(
                        out=out[pg : 2 * G : G, PRE:XL],
                        in_=stacc[pg][0:DW:32, PRE:XL],
                    )

            pend = None
            for g in range(G):
                for off, cw in CHUNKS_G[g]:
                    sl = slice(off, off + cw)
                    psq = []
                    for mt in range(MT):
                        pq = ps.tile([128, CHW], f32, tag="psq", name="psq", bufs=3)
                        zt = c77[:, g * M + mt * 128 : g * M + (mt + 1) * 128]
                        nc.tensor.matmul(pq[:, :cw], zt, xaug[:, sl])
                        psq.append(pq)
                    ktt = []
                    for mt in range(MT):
                        kt = work.tile([128, CHW], f16, tag=f"kt{mt}", name=f"kt{mt}")
                        nc.scalar.activation(
                            kt[:, :cw], psq[mt][:, :cw], EXP, scale=scale_imm
                        )
                        ktt.append(kt)
                    pr = {}
                    for mt, kt_ in TRI:
                        if mt not in pr:
                            pr[mt] = ps.tile(
                                [128, CHW], f32, tag="pr", name="pr", bufs=3
                            )
                        j = TRI.index((mt, kt_))
                        qt = qsb[:, Q_OFF[g] + j * 128 : Q_OFF[g] + (j + 1) * 128]
                        nc.tensor.matmul(
                            pr[mt][:, :cw],
                            qt,
                            ktt[kt_][:, :cw],
                            start=(kt_ == 0),
                            stop=(kt_ == mt),
                        )
                    if pend is not None:
                        emit_tail(pend)
                    pt = ps.tile([DW, CHW], f32, tag="po", name="po", bufs=2)
                    for mt in range(MT):
                        db = qsb[:, DV_OFF[g] + mt * DW : DV_OFF[g] + (mt + 1) * DW]
                        nc.tensor.matmul(
                            pt[0:DW, :cw],
                            db,
                            ktt[mt][:, :cw],
                            start=(mt == 0),
                            stop=(mt == MT - 1),
                        )
                    pk = []
                    for mt in range(MT):
                        k = work.tile([128, CHW], f16, tag=f"pk{mt}", name=f"pk{mt}")
                        nc.vector.tensor_mul(k[:, :cw], ktt[mt][:, :cw], pr[mt][:, :cw])
                        pk.append(k)
                    pa = work.tile([128, CHW], f16, tag="pa", name="pa")
                    nc.vector.tensor_add(pa[:, :cw], pk[0][:, :cw], pk[1][:, :cw])
                    pacc = work.tile([128, CHW], f16, tag="pacc", name="pacc")
                    nc.vector.tensor_add(pacc[:, :cw], pa[:, :cw], pk[2][:, :cw])
                    pend = (g, off, cw, pt, pacc)
            emit_tail(pend)

    nc.compile()
    return nc


def _get_nc(scale_imm: float):
    key = round(scale_imm, 12)
    if key not in _CACHE:
        _CACHE[key] = _build(scale_imm)
    return _CACHE[key]


def _host_prep(ND_X, Z, q_mu, q_sqrt, variance, lengthscale):
    from numpy.lib.stride_tricks import sliding_window_view

    ls = float(lengthscale)
    var = float(variance)
    scale = -0.5 / (ls * ls)
    f16 = np.float16

    x = np.asarray(ND_X, np.float32).reshape(N, H, W, C)
    swv = sliding_window_view(x, (PH, PW), axis=(1, 2))  # [N,28,28,C,5,5]
    pats = np.ascontiguousarray(swv.transpose(0, 1, 2, 4, 5, 3)).reshape(N, P, L)
    PNL = np.ascontiguousarray(pats.transpose(1, 0, 2))  # [P,N,L] float32

    Z64 = np.asarray(Z, np.float64)
    zsq = np.einsum("gml,gml->gm", Z64, Z64)  # [G,M]
    sqd = zsq[:, :, None] + zsq[:, None, :] - 2.0 * np.einsum(
        "gml,gnl->gmn", Z64, Z64
    )
    Kuu = var * np.exp(0.5 * sqd / (-ls * ls)) + JITTER * np.eye(M)
    Kinv = np.linalg.inv(Kuu)  # [G,M,M]
    Lq = np.tril(np.asarray(q_sqrt, np.float64))
    Bm = np.einsum("gmn,gnk->gmk", Kinv, Lq)
    Q = Kinv - np.einsum("gmk,gnk->gmn", Bm, Bm)  # [G,M,M]
    d = np.einsum("gmn,ng->gm", Kinv, np.asarray(q_mu, np.float64))  # [G,M]

    # qblob: 6 lower-tri blocks of Lh = 2*tril(Q,-1)+diag(Q), pre-transposed,
    # plus the [128, 33] d-blocks (d in col 0, zero pad) and the ones column.
    qblob = np.zeros([128, QB], f16)
    for g in range(G):
        Qs = 0.5 * (Q[g] + Q[g].T)
        Lh = 2.0 * np.tril(Qs, -1) + np.diag(np.diag(Qs))
        for j, (mt, kt_) in enumerate(TRI):
            blk = Lh[mt * 128 : (mt + 1) * 128, kt_ * 128 : (kt_ + 1) * 128]
            qblob[:, Q_OFF[g] + j * 128 : Q_OFF[g] + (j + 1) * 128] = blk.T.astype(
                f16
            )
        for mt in range(MT):
            qblob[:, DV_OFF[g] + mt * DW] = d[g, mt * 128 : (mt + 1) * 128].astype(
                f16
            )
    qblob[:, ONES_OFF] = f16(1.0)

    # blob77 shared part: zaug columns [LA, G*M]
    zaug = np.empty([LA, ZCOLS], f16)
    for g in range(G):
        zaug[:L, g * M : (g + 1) * M] = (-2.0 * Z64[g].T).astype(f16)
        zaug[L, g * M : (g + 1) * M] = (zsq[g] + np.log(var) / scale).astype(f16)
        zaug[L + 1, g * M : (g + 1) * M] = f16(1.0)

    in_maps = []
    for c in range(NCORES):
        Xc = PNL[c * PLOC : (c + 1) * PLOC].reshape(XL, L)  # [X, L] f32
        blob = np.empty([LA, B77], f16)
        blob[:, :ZCOLS] = zaug
        blob[:L, ZCOLS:] = Xc.T.astype(f16)
        blob[L, ZCOLS:] = f16(1.0)
        blob[L + 1, ZCOLS:] = np.einsum("xl,xl->x", Xc, Xc).astype(f16)
        in_maps.append({"blob77": blob, "qblob": qblob})
    return in_maps, scale, var


def _run(inputs, trace=False, trace_kwargs=None):
    _ensure_concourse()
    from concourse.bass_utils import run_bass_kernel_spmd

    in_maps, scale, var = _host_prep(**inputs)
    nc = _get_nc(scale)
    bkr = run_bass_kernel_spmd(
        nc,
        in_maps,
        list(range(NCORES)),
        trace=trace,
        **(trace_kwargs or {}),
    )
    mean = np.empty([N, P * G], np.float32)
    varr = np.empty([N, P * G], np.float32)
    for c in range(NCORES):
        o = np.asarray(bkr.results[c]["out"], np.float64)  # [2G, XL]
        m = o[:G]  # [G, XL]
        v = var - o[G:]
        mean[:, c * PLOC * G : (c + 1) * PLOC * G] = (
            m.reshape(G, PLOC, N).transpose(2, 1, 0).reshape(N, PLOC * G)
        )
        varr[:, c * PLOC * G : (c + 1) * PLOC * G] = (
            v.reshape(G, PLOC, N).transpose(2, 1, 0).reshape(N, PLOC * G)
        )
    return mean, varr, bkr


def kernel(**inputs):
    mean, varr, _ = _run(inputs, trace=False)
    return mean, varr


# revision 8
# speedup vs baseline: 1.1135x; 1.1135x over previous
"""Trainium2 Bass kernel for the patch-GP conditional (conv GP layer).

Contract: kernel(**inputs) takes the FULL inputs (as produced by
setup_inputs()) and returns the FULL output (mean, var), each [N, P*G].

Math (equivalent to the reference conditional):
    Kuf[g,m,x] = var * exp(scale*(||z_m||^2 - 2 z_m.x_x + ||x_x||^2)),
                 scale = -0.5/ls^2
    fmean[g,x] = d_g^T Kuf[g,:,x],          d_g  = Kuu_g^{-1} q_mu[:,g]
    fvar[g,x]  = var - Kuf^T Q_g Kuf,       Q_g  = Kuu_g^{-1} - B B^T,
                                            B    = Kuu_g^{-1} tril(q_sqrt_g)
Since Q is symmetric the quadratic form uses the halved-triangle trick:
    kt^T Q kt = kt^T Lh kt,   Lh = 2*tril(Q,-1) + diag(Q)
so only the 6 lower-triangular 128x128 blocks of Lh are multiplied
(6 PE streams instead of 9 per column chunk).

The per-inducing-point exp bias (scale*||z||^2 + ln var) and the per-column
||x||^2 are folded into two extra contraction rows of the sq matmul
(K = 75+2 = 77), so the ACT uses an immediate scale/zero bias and no
host-side rescale of the outputs is needed.

Per core (X = 98*32 = 3136 columns, chunks of 512 + a 64 tail), per g:
    psq_mt  = zaug_mt^T @ xaug     (3 matmuls, K=77, one PSUM bank each)
    kt_mt   = exp(scale*psq_mt)    (3 ACTs, fp16)
    R       = Lh @ kt              (6 matmuls - lower-tri blocks)
    pm      = dblk^T kt            (3 matmuls, M=33: d in col 0, zero pad
                                    so PSUM partitions 0..32 are defined)
    pk_mt   = kt_mt .* R_mt        (3 DVE muls)
    pa      = pk0 + pk1            (GpSimd add)   pacc = pa + pk2 (DVE)
    pv      = ones^T pacc          (1 matmul -> PSUM partition 32)
    copy    = PSUM[0:33] -> SBUF   (1 Scalar copy grabs pm and pv rows)
The pv matmul and the output copy of chunk i are emitted during chunk
i+1 (software pipelining) so the strict-FIFO PE/Scalar queues never
stall on the DVE/GpSimd reduction chain. Inputs arrive as packed blob
DMAs; outputs DMA from the staging rows (prefix early, 64-wide tail
last to keep the drain short).
"""

import numpy as np

# Problem constants (hardcoded per the task contract).
H = 32
W = 32
C = 3
PH = 5
PW = 5
JITTER = 1e-6
N = 32
G = 2
M = 384
L = PH * PW * C  # 75
LA = L + 2  # augmented contraction rows (ones, xsq)
P = (H - PH + 1) * (W - PW + 1)  # 784
NCORES = 8
PLOC = P // NCORES  # 98
XL = PLOC * N  # 3136
CHW = 512
_FULL = [(i * CHW, CHW) for i in range(XL // CHW)]
_TINY = [(XL - XL % CHW, XL % CHW)]
CHUNKS_G = [_FULL + _TINY, _FULL + _TINY]
PRE = XL - XL % CHW  # 3072: prefix DMA boundary
MT = M // 128  # 3 partition tiles of the inducing dim
TRI = [(0, 0), (1, 0), (1, 1), (2, 0), (2, 1), (2, 2)]  # (mt, kt) lower blocks
DW = 33  # pm lhsT width: d + zero pad covering PSUM partitions 0..32
WARM_MM = 14

# blob77 column layout (fp16): [zaug G*M | xt XL]
ZCOLS = G * M  # 768
B77 = ZCOLS + XL  # 3904
# qblob column layout (fp16), g0 essentials first:
#   [q-g0 6*128 | dblk-g0 3*33 | ones | q-g1 6*128 | dblk-g1 3*33]
QG = len(TRI) * 128  # 768
Q_OFF = [0, QG + MT * DW + 1]
DV_OFF = [QG, QG + MT * DW + 1 + QG]
ONES_OFF = QG + MT * DW
QB = 2 * QG + 2 * MT * DW + 1

_CACHE = {}


def _ensure_concourse():
    try:
        import concourse  # noqa: F401
    except ImportError:
        import sys

        for p in ("/opt/trn_rl_repo", "/root/.axon_site/_ro/trn_rl_repo"):
            if p not in sys.path:
                sys.path.insert(0, p)


def _build(scale_imm: float):
    """Build + compile the single-core SPMD program (same NEFF on all cores)."""
    _ensure_concourse()
    from concourse import bacc, mybir, tile

    f32 = mybir.dt.float32
    f16 = mybir.dt.float16
    EXP = mybir.ActivationFunctionType.Exp

    nc = bacc.Bacc("TRN2", target_bir_lowering=False, debug=False)

    blob77 = nc.dram_tensor("blob77", [LA, B77], f16, kind="ExternalInput").ap()
    qblob = nc.dram_tensor("qblob", [128, QB], f16, kind="ExternalInput").ap()
    out = nc.dram_tensor("out", [2 * G, XL], f32, kind="ExternalOutput").ap()

    with tile.TileContext(nc) as tc:
        with (
            tc.tile_pool(name="const", bufs=1) as const,
            tc.tile_pool(name="work", bufs=2) as work,
            tc.tile_pool(name="ps", bufs=1, space="PSUM") as ps,
        ):
            # PE warmup: dependency-free matmuls issued while input DMAs are
            # in flight so the HAM clock gate reaches 8/8 before real work.
            wsrc = const.tile([128, 256], f16)
            nc.vector.memset(wsrc, 0.0)
            for _ in range(WARM_MM):
                wps = ps.tile([128, CHW], f32, tag="pr", name="wps", bufs=3)
                nc.tensor.matmul(wps[:, 0:256], wsrc[:, 0:128], wsrc)

            c77 = const.tile([LA, B77], f16)
            nc.sync.dma_start(
                out=c77[:, 0 : ZCOLS + CHW], in_=blob77[:, 0 : ZCOLS + CHW]
            )
            qsb = const.tile([128, QB], f16)
            nc.sync.dma_start(out=qsb[:, 0 : Q_OFF[1]], in_=qblob[:, 0 : Q_OFF[1]])
            mid = ZCOLS + 3 * CHW
            nc.sync.dma_start(
                out=c77[:, ZCOLS + CHW : mid], in_=blob77[:, ZCOLS + CHW : mid]
            )
            nc.sync.dma_start(out=c77[:, mid:B77], in_=blob77[:, mid:B77])
            nc.sync.dma_start(out=qsb[:, Q_OFF[1] : QB], in_=qblob[:, Q_OFF[1] : QB])

            xaug = c77[:, ZCOLS:B77]
            ones = qsb[:, ONES_OFF : ONES_OFF + 1]

            stacc = [const.tile([DW, XL], f32, name=f"stacc{g}") for g in range(G)]

            def emit_tail(pend):
                """pv matmul + output copy for the previous chunk."""
                pg, poff, pcw, ppt, ppacc = pend
                psl = slice(poff, poff + pcw)
                nc.tensor.matmul(
                    ppt[32:33, :pcw], ones, ppacc[:, :pcw], skip_group_check=True
                )
                nc.scalar.copy(stacc[pg][:, psl], ppt[0:DW, :pcw])
                if poff + pcw == PRE:
                    nc.sync.dma_start(
                        out=out[pg : 2 * G : G, 0:PRE],
                        in_=stacc[pg][0:DW:32, 0:PRE],
                    )
                elif poff + pcw == XL:
                    nc.sync.dma_start(
                        out=out[pg : 2 * G : G, PRE:XL],
                        in_=stacc[pg][0:DW:32, PRE:XL],
                    )

            pend = None
            for g in range(G):
                for off, cw in CHUNKS_G[g]:
                    sl = slice(off, off + cw)
                    psq = []
                    for mt in range(MT):
                        pq = ps.tile([128, CHW], f32, tag="psq", name="psq", bufs=3)
                        zt = c77[:, g * M + mt * 128 : g * M + (mt + 1) * 128]
                        nc.tensor.matmul(pq[:, :cw], zt, xaug[:, sl])
                        psq.append(pq)
                    ktt = []
                    for mt in range(MT):
                        kt = work.tile([128, CHW], f16, tag=f"kt{mt}", name=f"kt{mt}")
                        nc.scalar.activation(
                            kt[:, :cw], psq[mt][:, :cw], EXP, scale=scale_imm
                        )
                        ktt.append(kt)
                    pr = {}
                    for mt, kt_ in TRI:
                        if mt not in pr:
                            pr[mt] = ps.tile(
                                [128, CHW], f32, tag="pr", name="pr", bufs=3
                            )
                        j = TRI.index((mt, kt_))
                        qt = qsb[:, Q_OFF[g] + j * 128 : Q_OFF[g] + (j + 1) * 128]
                        nc.tensor.matmul(
                            pr[mt][:, :cw],
                            qt,
                            ktt[kt_][:, :cw],
                            start=(kt_ == 0),
                            stop=(kt_ == mt),
                        )
                    if pend is not None:
                        emit_tail(pend)
                    pt = ps.tile([DW, CHW], f32, tag="po", name="po", bufs=2)
                    for mt in range(MT):
                        db = qsb[:, DV_OFF[g] + mt * DW : DV_OFF[g] + (mt + 1) * DW]
                        nc.tensor.matmul(
                            pt[0:DW, :cw],
                            db,
                            ktt[mt][:, :cw],
                            start=(mt == 0),
                            stop=(mt == MT - 1),
                        )
                    pk = []
                    for mt in range(MT):
                        k = work.tile([128, CHW], f16, tag=f"pk{mt}", name=f"pk{mt}")
                        nc.vector.tensor_mul(k[:, :cw], ktt[mt][:, :cw], pr[mt][:, :cw])
                        pk.append(k)
                    pa = work.tile([128, CHW], f16, tag="pa", name="pa")
                    nc.vector.tensor_add(pa[:, :cw], pk[0][:, :cw], pk[1][:, :cw])
                    pacc = work.tile([128, CHW], f16, tag="pacc", name="pacc")
                    nc.vector.tensor_add(pacc[:, :cw], pa[:, :cw], pk[2][:, :cw])
                    pend = (g, off, cw, pt, pacc)
            emit_tail(pend)

    nc.compile()
    return nc


def _get_nc(scale_imm: float):
    key = round(scale_imm, 12)
    if key not in _CACHE:
        _CACHE[key] = _build(scale_imm)
    return _CACHE[key]


def _host_prep(ND_X, Z, q_mu, q_sqrt, variance, lengthscale):
    from numpy.lib.stride_tricks import sliding_window_view

    ls = float(lengthscale)
    var = float(variance)
    scale = -0.5 / (ls * ls)
    f16 = np.float16

    x = np.asarray(ND_X, np.float32).reshape(N, H, W, C)
    swv = sliding_window_view(x, (PH, PW), axis=(1, 2))  # [N,28,28,C,5,5]
    pats = np.ascontiguousarray(swv.transpose(0, 1, 2, 4, 5, 3)).reshape(N, P, L)
    PNL = np.ascontiguousarray(pats.transpose(1, 0, 2))  # [P,N,L] float32

    Z64 = np.asarray(Z, np.float64)
    zsq = np.einsum("gml,gml->gm", Z64, Z64)  # [G,M]
    sqd = zsq[:, :, None] + zsq[:, None, :] - 2.0 * np.einsum(
        "gml,gnl->gmn", Z64, Z64
    )
    Kuu = var * np.exp(0.5 * sqd / (-ls * ls)) + JITTER * np.eye(M)
    Kinv = np.linalg.inv(Kuu)  # [G,M,M]
    Lq = np.tril(np.asarray(q_sqrt, np.float64))
    Bm = np.einsum("gmn,gnk->gmk", Kinv, Lq)
    Q = Kinv - np.einsum("gmk,gnk->gmn", Bm, Bm)  # [G,M,M]
    d = np.einsum("gmn,ng->gm", Kinv, np.asarray(q_mu, np.float64))  # [G,M]

    # qblob: 6 lower-tri blocks of Lh = 2*tril(Q,-1)+diag(Q), pre-transposed,
    # plus the [128, 33] d-blocks (d in col 0, zero pad) and the ones column.
    qblob = np.zeros([128, QB], f16)
    for g in range(G):
        Qs = 0.5 * (Q[g] + Q[g].T)
        Lh = 2.0 * np.tril(Qs, -1) + np.diag(np.diag(Qs))
        for j, (mt, kt_) in enumerate(TRI):
            blk = Lh[mt * 128 : (mt + 1) * 128, kt_ * 128 : (kt_ + 1) * 128]
            qblob[:, Q_OFF[g] + j * 128 : Q_OFF[g] + (j + 1) * 128] = blk.T.astype(
                f16
            )
        for mt in range(MT):
            qblob[:, DV_OFF[g] + mt * DW] = d[g, mt * 128 : (mt + 1) * 128].astype(
                f16
            )
    qblob[:, ONES_OFF] = f16(1.0)

    # blob77 shared part: zaug columns [LA, G*M]
    zaug = np.empty([LA, ZCOLS], f16)
    for g in range(G):
        zaug[:L, g * M : (g + 1) * M] = (-2.0 * Z64[g].T).astype(f16)
        zaug[L, g * M : (g + 1) * M] = (zsq[g] + np.log(var) / scale).astype(f16)
        zaug[L + 1, g * M : (g + 1) * M] = f16(1.0)

    in_maps = []
    for c in range(NCORES):
        Xc = PNL[c * PLOC : (c + 1) * PLOC].reshape(XL, L)  # [X, L] f32
        blob = np.empty([LA, B77], f16)
        blob[:, :ZCOLS] = zaug
        blob[:L, ZCOLS:] = Xc.T.astype(f16)
        blob[L, ZCOLS:] = f16(1.0)
        blob[L + 1, ZCOLS:] = np.einsum("xl,xl->x", Xc, Xc).astype(f16)
        in_maps.append({"blob77": blob, "qblob": qblob})
    return in_maps, scale, var


def _run(inputs, trace=False, trace_kwargs=None):
    _ensure_concourse()
    from concourse.bass_utils import run_bass_kernel_spmd

    in_maps, scale, var = _host_prep(**inputs)
    nc = _get_nc(scale)
    bkr = run_bass_kernel_spmd(
        nc,
        in_maps,
        list(range(NCORES)),
        trace=trace,
        **(trace_kwargs or {}),
    )
    mean = np.empty([N, P * G], np.float32)
    varr = np.empty([N, P * G], np.float32)
    for c in range(NCORES):
        o = np.asarray(bkr.results[c]["out"], np.float64)  # [2G, XL]
        m = o[:G]  # [G, XL]
        v = var - o[G:]
        mean[:, c * PLOC * G : (c + 1) * PLOC * G] = (
            m.reshape(G, PLOC, N).transpose(2, 1, 0).reshape(N, PLOC * G)
        )
        varr[:, c * PLOC * G : (c + 1) * PLOC * G] = (
            v.reshape(G, PLOC, N).transpose(2, 1, 0).reshape(N, PLOC * G)
        )
    return mean, varr, bkr


def kernel(**inputs):
    mean, varr, _ = _run(inputs, trace=False)
    return mean, varr


# revision 9
# speedup vs baseline: 1.1930x; 1.0714x over previous
"""Trainium2 Bass kernel for the patch-GP conditional (conv GP layer).

Contract: kernel(**inputs) takes the FULL inputs (as produced by
setup_inputs()) and returns the FULL output (mean, var), each [N, P*G].

Math (equivalent to the reference conditional):
    Kuf[g,m,x] = var * exp(scale*(||z_m||^2 - 2 z_m.x_x + ||x_x||^2)),
                 scale = -0.5/ls^2
    fmean[g,x] = d_g^T Kuf[g,:,x],          d_g  = Kuu_g^{-1} q_mu[:,g]
    fvar[g,x]  = var - Kuf^T Q_g Kuf,       Q_g  = Kuu_g^{-1} - B B^T,
                                            B    = Kuu_g^{-1} tril(q_sqrt_g)
Since Q is symmetric the quadratic form uses the halved-triangle trick:
    kt^T Q kt = kt^T Lh kt,   Lh = 2*tril(Q,-1) + diag(Q)
so only the 6 lower-triangular 128x128 blocks of Lh are multiplied
(6 PE streams instead of 9 per column chunk).

The per-inducing-point exp bias (scale*||z||^2 + ln var) and the per-column
||x||^2 are folded into two extra contraction rows of the sq matmul
(K = 75+2 = 77), so the ACT uses an immediate scale/zero bias and no
host-side rescale of the outputs is needed.

Per core (X = 98*32 = 3136 columns, chunks of 512 + a 64 tail), per g:
    psq_mt  = zaug_mt^T @ xaug     (3 matmuls, K=77, one PSUM bank each)
    kt_mt   = exp(scale*psq_mt)    (3 ACTs, fp16)
    R       = Lh @ kt              (6 matmuls - lower-tri blocks)
    pm      = dblk^T kt            (3 matmuls, M=33: d in col 0, zero pad
                                    so PSUM partitions 0..32 are defined)
    pk_mt   = kt_mt .* R_mt        (3 DVE muls)
    pa      = pk0 + pk1            (GpSimd add)   pacc = pa + pk2 (DVE)
    pv      = ones^T pacc          (1 matmul -> PSUM partition 32)
    copy    = PSUM[0:33] -> SBUF   (1 Scalar copy grabs pm and pv rows)
The pv matmul and the output copy of chunk i are emitted during chunk
i+1 (software pipelining) so the strict-FIFO PE/Scalar queues never
stall on the DVE/GpSimd reduction chain. Inputs arrive as packed blob
DMAs; outputs DMA from the staging rows (prefix early, 64-wide tail
last to keep the drain short).
"""

import numpy as np

# Problem constants (hardcoded per the task contract).
H = 32
W = 32
C = 3
PH = 5
PW = 5
JITTER = 1e-6
N = 32
G = 2
M = 384
L = PH * PW * C  # 75
LA = L + 2  # augmented contraction rows (ones, xsq)
P = (H - PH + 1) * (W - PW + 1)  # 784
NCORES = 8
PLOC = P // NCORES  # 98
XL = PLOC * N  # 3136
CHW = 512
_FULL = [(i * CHW, CHW) for i in range(XL // CHW)]
_TINY = [(XL - XL % CHW, XL % CHW)]
CHUNKS_G = [_FULL + _TINY, _FULL + _TINY]
PRE = XL - XL % CHW  # 3072: prefix DMA boundary
MT = M // 128  # 3 partition tiles of the inducing dim
TRI = [(0, 0), (1, 0), (1, 1), (2, 0), (2, 1), (2, 2)]  # (mt, kt) lower blocks
DW = 33  # pm lhsT width: d + zero pad covering PSUM partitions 0..32
WARM_MM = 14

# blob77 column layout (fp16): [zaug G*M | xt XL]
ZCOLS = G * M  # 768
B77 = ZCOLS + XL  # 3904
# qblob column layout (fp16), g0 essentials first:
#   [q-g0 6*128 | dblk-g0 3*33 | ones | q-g1 6*128 | dblk-g1 3*33]
QG = len(TRI) * 128  # 768
Q_OFF = [0, QG + MT * DW + 1]
DV_OFF = [QG, QG + MT * DW + 1 + QG]
ONES_OFF = QG + MT * DW
QB = 2 * QG + 2 * MT * DW + 1

_CACHE = {}


def _ensure_concourse():
    try:
        import concourse  # noqa: F401
    except ImportError:
        import sys

        for p in ("/opt/trn_rl_repo", "/root/.axon_site/_ro/trn_rl_repo"):
            if p not in sys.path:
                sys.path.insert(0, p)


def _build(scale_imm: float):
    """Build + compile the single-core SPMD program (same NEFF on all cores)."""
    _ensure_concourse()
    from concourse import bacc, mybir, tile

    f32 = mybir.dt.float32
    f16 = mybir.dt.float16
    EXP = mybir.ActivationFunctionType.Exp

    nc = bacc.Bacc("TRN2", target_bir_lowering=False, debug=False)

    blob77 = nc.dram_tensor("blob77", [LA, B77], f16, kind="ExternalInput").ap()
    qblob = nc.dram_tensor("qblob", [128, QB], f16, kind="ExternalInput").ap()
    out = nc.dram_tensor("out", [2 * G, XL], f32, kind="ExternalOutput").ap()

    with tile.TileContext(nc) as tc:
        with (
            tc.tile_pool(name="const", bufs=1) as const,
            tc.tile_pool(name="work", bufs=2) as work,
            tc.tile_pool(name="ps", bufs=1, space="PSUM") as ps,
        ):
            # PE warmup: dependency-free matmuls issued while input DMAs are
            # in flight so the HAM clock gate reaches 8/8 before real work.
            wsrc = const.tile([128, 256], f16)
            nc.vector.memset(wsrc, 0.0)
            for _ in range(WARM_MM):
                wps = ps.tile([128, CHW], f32, tag="pr", name="wps", bufs=4)
                nc.tensor.matmul(wps[:, 0:256], wsrc[:, 0:128], wsrc)

            c77 = const.tile([LA, B77], f16)
            nc.sync.dma_start(
                out=c77[:, 0 : ZCOLS + CHW], in_=blob77[:, 0 : ZCOLS + CHW]
            )
            qsb = const.tile([128, QB], f16)
            nc.sync.dma_start(out=qsb[:, 0 : Q_OFF[1]], in_=qblob[:, 0 : Q_OFF[1]])
            mid = ZCOLS + 3 * CHW
            nc.sync.dma_start(
                out=c77[:, ZCOLS + CHW : mid], in_=blob77[:, ZCOLS + CHW : mid]
            )
            nc.sync.dma_start(out=c77[:, mid:B77], in_=blob77[:, mid:B77])
            nc.sync.dma_start(out=qsb[:, Q_OFF[1] : QB], in_=qblob[:, Q_OFF[1] : QB])

            xaug = c77[:, ZCOLS:B77]
            ones = qsb[:, ONES_OFF : ONES_OFF + 1]

            stacc = [const.tile([DW, XL], f32, name=f"stacc{g}") for g in range(G)]

            def emit_tail(pend):
                """pv matmul + output copy for the previous chunk."""
                pg, poff, pcw, ppt, ppacc = pend
                psl = slice(poff, poff + pcw)
                nc.tensor.matmul(
                    ppt[32:33, :pcw], ones, ppacc[:, :pcw], skip_group_check=True
                )
                nc.scalar.copy(stacc[pg][:, psl], ppt[0:DW, :pcw])
                if poff + pcw == PRE:
                    nc.sync.dma_start(
                        out=out[pg : 2 * G : G, 0:PRE],
                        in_=stacc[pg][0:DW:32, 0:PRE],
                    )
                elif poff + pcw == XL:
                    nc.sync.dma_start(
                        out=out[pg : 2 * G : G, PRE:XL],
                        in_=stacc[pg][0:DW:32, PRE:XL],
                    )

            pend = None
            for g in range(G):
                for off, cw in CHUNKS_G[g]:
                    sl = slice(off, off + cw)
                    psq = []
                    for mt in range(MT):
                        pq = ps.tile([128, CHW], f32, tag="psq", name="psq", bufs=3)
                        zt = c77[:, g * M + mt * 128 : g * M + (mt + 1) * 128]
                        nc.tensor.matmul(pq[:, :cw], zt, xaug[:, sl])
                        psq.append(pq)
                    ktt = []
                    for mt in range(MT):
                        kt = work.tile([128, CHW], f16, tag=f"kt{mt}", name=f"kt{mt}")
                        nc.scalar.activation(
                            kt[:, :cw], psq[mt][:, :cw], EXP, scale=scale_imm
                        )
                        ktt.append(kt)
                    pr = {}
                    for mt, kt_ in TRI:
                        if mt not in pr:
                            pr[mt] = ps.tile(
                                [128, CHW], f32, tag="pr", name="pr", bufs=4
                            )
                        j = TRI.index((mt, kt_))
                        qt = qsb[:, Q_OFF[g] + j * 128 : Q_OFF[g] + (j + 1) * 128]
                        nc.tensor.matmul(
                            pr[mt][:, :cw],
                            qt,
                            ktt[kt_][:, :cw],
                            start=(kt_ == 0),
                            stop=(kt_ == mt),
                        )
                    if pend is not None:
                        emit_tail(pend)
                    pt = ps.tile([DW, CHW], f32, tag="po", name="po", bufs=1)
                    for mt in range(MT):
                        db = qsb[:, DV_OFF[g] + mt * DW : DV_OFF[g] + (mt + 1) * DW]
                        nc.tensor.matmul(
                            pt[0:DW, :cw],
                            db,
                            ktt[mt][:, :cw],
                            start=(mt == 0),
                            stop=(mt == MT - 1),
                        )
                    pk = []
                    for mt in range(MT):
                        k = work.tile([128, CHW], f16, tag=f"pk{mt}", name=f"pk{mt}")
                        nc.vector.tensor_mul(k[:, :cw], ktt[mt][:, :cw], pr[mt][:, :cw])
                        pk.append(k)
                    pa = work.tile([128, CHW], f16, tag="pa", name="pa")
                    nc.vector.tensor_add(pa[:, :cw], pk[0][:, :cw], pk[1][:, :cw])
                    pacc = work.tile([128, CHW], f16, tag="pacc", name="pacc")
                    nc.vector.tensor_add(pacc[:, :cw], pa[:, :cw], pk[2][:, :cw])
                    pend = (g, off, cw, pt, pacc)
            emit_tail(pend)

    nc.compile()
    return nc


def _get_nc(scale_imm: float):
    key = round(scale_imm, 12)
    if key not in _CACHE:
        _CACHE[key] = _build(scale_imm)
    return _CACHE[key]


def _host_prep(ND_X, Z, q_mu, q_sqrt, variance, lengthscale):
    from numpy.lib.stride_tricks import sliding_window_view

    ls = float(lengthscale)
    var = float(variance)
    scale = -0.5 / (ls * ls)
    f16 = np.float16

    x = np.asarray(ND_X, np.float32).reshape(N, H, W, C)
    swv = sliding_window_view(x, (PH, PW), axis=(1, 2))  # [N,28,28,C,5,5]
    pats = np.ascontiguousarray(swv.transpose(0, 1, 2, 4, 5, 3)).reshape(N, P, L)
    PNL = np.ascontiguousarray(pats.transpose(1, 0, 2))  # [P,N,L] float32

    Z64 = np.asarray(Z, np.float64)
    zsq = np.einsum("gml,gml->gm", Z64, Z64)  # [G,M]
    sqd = zsq[:, :, None] + zsq[:, None, :] - 2.0 * np.einsum(
        "gml,gnl->gmn", Z64, Z64
    )
    Kuu = var * np.exp(0.5 * sqd / (-ls * ls)) + JITTER * np.eye(M)
    Kinv = np.linalg.inv(Kuu)  # [G,M,M]
    Lq = np.tril(np.asarray(q_sqrt, np.float64))
    Bm = np.einsum("gmn,gnk->gmk", Kinv, Lq)
    Q = Kinv - np.einsum("gmk,gnk->gmn", Bm, Bm)  # [G,M,M]
    d = np.einsum("gmn,ng->gm", Kinv, np.asarray(q_mu, np.float64))  # [G,M]

    # qblob: 6 lower-tri blocks of Lh = 2*tril(Q,-1)+diag(Q), pre-transposed,
    # plus the [128, 33] d-blocks (d in col 0, zero pad) and the ones column.
    qblob = np.zeros([128, QB], f16)
    for g in range(G):
        Qs = 0.5 * (Q[g] + Q[g].T)
        Lh = 2.0 * np.tril(Qs, -1) + np.diag(np.diag(Qs))
        for j, (mt, kt_) in enumerate(TRI):
            blk = Lh[mt * 128 : (mt + 1) * 128, kt_ * 128 : (kt_ + 1) * 128]
            qblob[:, Q_OFF[g] + j * 128 : Q_OFF[g] + (j + 1) * 128] = blk.T.astype(
                f16
            )
        for mt in range(MT):
            qblob[:, DV_OFF[g] + mt * DW] = d[g, mt * 128 : (mt + 1) * 128].astype(
                f16
            )
    qblob[:, ONES_OFF] = f16(1.0)

    # blob77 shared part: zaug columns [LA, G*M]
    zaug = np.empty([LA, ZCOLS], f16)
    for g in range(G):
        zaug[:L, g * M : (g + 1) * M] = (-2.0 * Z64[g].T).astype(f16)
        zaug[L, g * M : (g + 1) * M] = (zsq[g] + np.log(var) / scale).astype(f16)
        zaug[L + 1, g * M : (g + 1) * M] = f16(1.0)

    in_maps = []
    for c in range(NCORES):
        Xc = PNL[c * PLOC : (c + 1) * PLOC].reshape(XL, L)  # [X, L] f32
        blob = np.empty([LA, B77], f16)
        blob[:, :ZCOLS] = zaug
        blob[:L, ZCOLS:] = Xc.T.astype(f16)
        blob[L, ZCOLS:] = f16(1.0)
        blob[L + 1, ZCOLS:] = np.einsum("xl,xl->x", Xc, Xc).astype(f16)
        in_maps.append({"blob77": blob, "qblob": qblob})
    return in_maps, scale, var


def _run(inputs, trace=False, trace_kwargs=None):
    _ensure_concourse()
    from concourse.bass_utils import run_bass_kernel_spmd

    in_maps, scale, var = _host_prep(**inputs)
    nc = _get_nc(scale)
    bkr = run_bass_kernel_spmd(
        nc,
        in_maps,
        list(range(NCORES)),
        trace=trace,
        **(trace_kwargs or {}),
    )
    mean = np.empty([N, P * G], np.float32)
    varr = np.empty([N, P * G], np.float32)
    for c in range(NCORES):
        o = np.asarray(bkr.results[c]["out"], np.float64)  # [2G, XL]
        m = o[:G]  # [G, XL]
        v = var - o[G:]
        mean[:, c * PLOC * G : (c + 1) * PLOC * G] = (
            m.reshape(G, PLOC, N).transpose(2, 1, 0).reshape(N, PLOC * G)
        )
        varr[:, c * PLOC * G : (c + 1) * PLOC * G] = (
            v.reshape(G, PLOC, N).transpose(2, 1, 0).reshape(N, PLOC * G)
        )
    return mean, varr, bkr


def kernel(**inputs):
    mean, varr, _ = _run(inputs, trace=False)
    return mean, varr
